# revision 1
# baseline (speedup 1.0000x reference)
"""GAT layer (nn_GATNode) Trainium2 Bass kernel.

Sharding: 8 cores; core c handles batch b = c//2 and the head pair
hp = c%2 (heads 2*hp, 2*hp+1) over the full 2048x2048 attention matrix.

Math (per batch b, head h):
  e1[i] = h_t[i] @ (Wq[h] @ a1[h]),  e2[j] = h_t[j] @ (Wk[h] @ a2[h])
  exp(lrelu(e1+e2)) = max(exp(e1+e2), exp(0.2*(e1+e2)))
  Dividing softmax column j by exp(0.2*e2[j]) (cancels in softmax):
    Utilde[i,j] = max(f1[i]*r[j], g1[i]),
      f1 = exp(e1), g1 = exp(0.2*e1), r = exp(0.8*e2)
  A[i,j] = adj[i,j] * Utilde[i,j]          (one 4x tensor_scalar + one 2x
                                            tensor_tensor per tile)
  S[j] = sum_i A[i,j]                       (PE ones-matmul, natural layout)
  h'[i,:] = sum_j A[i,j] * Wh[j,:]/S[j]     (PE matmul on xbar-transposed A)
  out = ELU(h')
"""

import os
import numpy as np

B, N, F, T, H, D = 4, 2048, 256, 8, 4, 64
FT = F + T          # 264
FTP = 384           # padded to 3*128 for the xbar transpose
NT = N // 128       # 16 node tiles
FC = 3              # f chunks of 128 (264 -> 384)

_CACHE = {}


def _build_program():
    import concourse.bass as bass
    import concourse.bacc as bacc
    import concourse.mybir as mybir
    from concourse import tile

    F32 = mybir.dt.float32
    F16 = mybir.dt.float16
    I32 = mybir.dt.int32
    AL = mybir.AluOpType
    ACT = mybir.ActivationFunctionType

    nc = bacc.Bacc("TRN2", target_bir_lowering=False, debug=False)

    adj_d = nc.dram_tensor("adj", [N, N], I32, kind="ExternalInput").ap()
    x_d = nc.dram_tensor("x", [N, F], F32, kind="ExternalInput").ap()
    toh_d = nc.dram_tensor("toh", [N, T], F32, kind="ExternalInput").ap()
    wq_d = nc.dram_tensor("wq2", [2, FT, D], F32, kind="ExternalInput").ap()
    wk_d = nc.dram_tensor("wk2", [2, FT, D], F32, kind="ExternalInput").ap()
    w_d = nc.dram_tensor("w2", [2, F, D], F32, kind="ExternalInput").ap()
    a_d = nc.dram_tensor("a2", [2, 2 * D, 1], F32, kind="ExternalInput").ap()
    out_d = nc.dram_tensor("out", [N, 2 * D], F32, kind="ExternalOutput").ap()
    s_scr = [nc.dram_tensor(f"s_scr{h}", [1, N], F32, kind="Internal").ap()
             for h in range(2)]

    with tile.TileContext(nc) as tc:
        with (
            tc.tile_pool(name="persist", bufs=1) as pp,
            tc.tile_pool(name="psS", bufs=1, space="PSUM") as ps_S,
        ):
            # ---- persistent SBUF (atT0/atT1 64KB/part each) ----
            atT = [pp.tile([128, NT * N], F16, tag=f"atT{h}", name=f"atT{h}")
                   for h in range(2)]
            out_sb = pp.tile([128, NT * 128], F32, tag="outsb")  # 8KB/part
            r_bc = [pp.tile([128, N], F16, tag=f"rbc{h}", name=f"rbc{h}")
                    for h in range(2)]
            wh = [pp.tile([128, NT * D], F16, tag=f"wh{h}", name=f"wh{h}")
                  for h in range(2)]
            f_cols = [pp.tile([128, NT], F32, tag=f"fc{h}", name=f"fc{h}")
                      for h in range(2)]
            g_cols = [pp.tile([128, NT], F32, tag=f"gc{h}", name=f"gc{h}")
                      for h in range(2)]
            vst = pp.tile([128, FC * 4], F32, tag="vst")
            ones1 = pp.tile([128, 1], F16, tag="ones1")
            ones128 = pp.tile([128, 128], F16, tag="ones128")
            nc.vector.memset(ones1[:], 1.0)
            nc.vector.memset(ones128[:], 1.0)

            # ---- phase A: h_t^T via xbar transpose; e-vectors; Wh ----
            with (
                tc.tile_pool(name="phA", bufs=2) as pa,
                tc.tile_pool(name="phAps", bufs=2, space="PSUM") as pap,
            ):
                htT = pa.tile([128, FC * N], F16, tag="htT", bufs=1)
                # prefetch attention-vector weights (issued after the critical-
                # path x load so Pool-FIFO emissions don't delay it)
                wall = pa.tile([128, 12 * D], F32, tag="wall", bufs=1)
                nc.vector.memset(wall[:], 0.0)
                abc_t = []
                for hh in range(2):
                    a1bc = pa.tile([128, D], F32, tag=f"a1bc{hh}", name=f"a1bc{hh}", bufs=1)
                    a2bc = pa.tile([128, D], F32, tag=f"a2bc{hh}", name=f"a2bc{hh}", bufs=1)
                    nc.gpsimd.dma_start(
                        a1bc[:],
                        a_d[hh, 0:D, :].rearrange("a b -> b a").broadcast_to((128, D)))
                    nc.gpsimd.dma_start(
                        a2bc[:],
                        a_d[hh, D:2 * D, :].rearrange("a b -> b a").broadcast_to((128, D)))
                    abc_t.append((a1bc, a2bc))
                    for ft in range(FC):
                        for qk, wsrc in enumerate((wq_d, wk_d)):
                            ci = (hh * FC + ft) * 2 + qk
                            lo = ft * 128
                            hi = min(FT, lo + 128)
                            nc.gpsimd.dma_start(
                                wall[:hi - lo, ci * D:(ci + 1) * D],
                                wsrc[hh, lo:hi, :])

                # batched x/toh load + strided casts into padded h_t layout
                xall = pa.tile([128, NT * F], F32, tag="xall", bufs=1)
                tohall = pa.tile([128, NT * T], F32, tag="tohall", bufs=1)
                ht_all = pa.tile([128, NT * FTP], F16, tag="ht_all", bufs=1)
                nc.gpsimd.dma_start(
                    xall[:].rearrange("p (t c) -> p t c", t=NT),
                    x_d[:].rearrange("(t p) c -> p t c", p=128))
                nc.gpsimd.dma_start(
                    tohall[:].rearrange("p (t c) -> p t c", t=NT),
                    toh_d[:].rearrange("(t p) c -> p t c", p=128))
                nc.gpsimd.memset(
                    ht_all[:].rearrange("p (t c) -> p t c", c=FTP)[:, :, FT:], 0.0)
                nc.scalar.copy(
                    ht_all[:].rearrange("p (t c) -> p t c", c=FTP)[:, :, 0:F],
                    xall[:].rearrange("p (t c) -> p t c", c=F))
                nc.vector.tensor_copy(
                    ht_all[:].rearrange("p (t c) -> p t c", c=FTP)[:, :, F:FT],
                    tohall[:].rearrange("p (t c) -> p t c", c=T))
                with tc.high_priority():
                    for it in range(NT):
                        nc.sync.dma_start_transpose(
                            htT[:].rearrange("p (t c) -> p t c", t=FC)[
                                :, :, it * 128:(it + 1) * 128],
                            ht_all[:, it * FTP:(it + 1) * FTP],
                        )

                # V vectors (Wq @ a1, Wk @ a2) via STT row-dots
                for hh in range(2):
                    a1bc, a2bc = abc_t[hh]
                    for ft in range(FC):
                        for qk, abc in enumerate((a1bc, a2bc)):
                            ci = (hh * FC + ft) * 2 + qk
                            junk = pa.tile([128, D], F32, tag="vjunk")
                            nc.vector.scalar_tensor_tensor(
                                junk[:], wall[:, ci * D:(ci + 1) * D], 1.0, abc[:],
                                AL.mult, AL.mult,
                                accum_out=vst[:, ft * 4 + 2 * hh + qk: ft * 4 + 2 * hh + qk + 1])

                vst16 = pa.tile([128, FC * 4], F16, tag="vst16")
                nc.vector.tensor_copy(vst16[:], vst[:])

                # e-columns [node, vec] accumulated over f-chunks
                e_cols = pap.tile([128, NT * 4], F32, tag="ecols", bufs=1)
                for it in range(NT):
                    for ft in range(FC):
                        nc.tensor.matmul(
                            e_cols[:, it * 4:(it + 1) * 4],
                            htT[:, ft * N + it * 128: ft * N + (it + 1) * 128],
                            vst16[:, ft * 4:(ft + 1) * 4],
                            start=(ft == 0), stop=(ft == FC - 1))
                for hh in range(2):
                    ecol_h = e_cols[:].rearrange("p (t v) -> p t v", v=4)[
                        :, :, 2 * hh:2 * hh + 1]
                    nc.scalar.activation(f_cols[hh][:], ecol_h, ACT.Exp, scale=1.0)
                    nc.scalar.activation(g_cols[hh][:], ecol_h, ACT.Exp, scale=0.2)

                # r_bc per head: e2 broadcast across partitions, exp(0.8*)
                for hh in range(2):
                    for jc in range(4):
                        psr = pap.tile([128, 512], F32, tag="psr", bufs=1)
                        for ft in range(FC):
                            vkbc = pa.tile([128, 128], F16, tag="vkbc")
                            nc.vector.tensor_scalar(
                                vkbc[:], ones128[:],
                                vst[:, ft * 4 + 2 * hh + 1: ft * 4 + 2 * hh + 2],
                                None, AL.mult)
                            nc.tensor.matmul(
                                psr[:],
                                vkbc[:],
                                htT[:, ft * N + jc * 512: ft * N + (jc + 1) * 512],
                                start=(ft == 0), stop=(ft == FC - 1))
                        nc.scalar.activation(
                            r_bc[hh][:, jc * 512:(jc + 1) * 512], psr[:],
                            ACT.Exp, scale=0.8)

                # Wh per head
                for hh in range(2):
                    wb = []
                    for ft in range(2):
                        wstg2 = pa.tile([128, D], F32, tag="wstg2")
                        nc.gpsimd.dma_start(wstg2[:], w_d[hh, ft * 128:(ft + 1) * 128, :])
                        wb16 = pa.tile([128, D], F16, tag=f"wb16_{hh}_{ft}",
                                       name=f"wb16_{hh}_{ft}")
                        nc.vector.tensor_copy(wb16[:], wstg2[:])
                        wb.append(wb16)
                    for it in range(NT):
                        pswh = pap.tile([128, D], F32, tag="pswh")
                        for ft in range(2):
                            nc.tensor.matmul(
                                pswh[:],
                                htT[:, ft * N + it * 128: ft * N + (it + 1) * 128],
                                wb[ft][:],
                                start=(ft == 0), stop=(ft == 1))
                        nc.vector.tensor_copy(
                            wh[hh][:, it * D:(it + 1) * D], pswh[:])

            # ---- big loop: both heads interleaved per i-tile ----
            # both heads' column sums in one 4-bank psum [2, 2048]
            sS = ps_S.tile([64, N], F32, tag="sS")
            with tc.tile_pool(name="bigloop", bufs=3) as pb:
                for it in range(NT):
                    stg = pb.tile([128, N], I32, tag="stg", bufs=2)
                    nc.gpsimd.dma_start(stg[:], adj_d[it * 128:(it + 1) * 128, :])
                    adjt = pb.tile([128, N], F16, tag="adjt", bufs=3)
                    nc.scalar.copy(adjt[:], stg[:])
                    for hh in range(2):
                        u = pb.tile([128, N], F16, tag="u", bufs=2)
                        nc.vector.tensor_scalar(
                            u[:], r_bc[hh][:],
                            f_cols[hh][:, it:it + 1], g_cols[hh][:, it:it + 1],
                            AL.mult, AL.max)
                        at = pb.tile([128, N], F16, tag="at", bufs=5)
                        nc.vector.tensor_tensor(at[:], u[:], adjt[:], AL.mult)
                        for jc in range(4):
                            nc.tensor.matmul(
                                sS[32 * hh:32 * hh + 1, jc * 512:(jc + 1) * 512],
                                ones1[:],
                                at[:, jc * 512:(jc + 1) * 512],
                                start=(it == 0), stop=(it == NT - 1))
                        nc.sync.dma_start_transpose(
                            atT[hh][:].rearrange("p (t c) -> p t c", t=NT)[
                                :, :, it * 128:(it + 1) * 128],
                            at[:])

            # ---- normalize + main matmuls + ELU ----
            with (
                tc.tile_pool(name="post", bufs=1) as po,
                tc.tile_pool(name="psO", bufs=2, space="PSUM") as ps_O,
            ):
                for hh in range(2):
                    s_row = po.tile([1, N], F32, tag="srow", bufs=1)
                    nc.vector.tensor_scalar_add(s_row[:], sS[32 * hh:32 * hh + 1, :], 1e-30)
                    nc.gpsimd.dma_start(s_scr[hh][:], s_row[:])
                    s_sp = po.tile([128, NT], F32, tag="ssp", bufs=2)
                    nc.gpsimd.dma_start(
                        s_sp[:],
                        s_scr[hh][:].rearrange("o (t p) -> (o p) t", p=128))
                    s_rec = po.tile([128, NT], F32, tag="srec", bufs=2)
                    nc.vector.reciprocal(s_rec[:], s_sp[:])
                    whp = po.tile([128, NT * D], F16, tag="whp", bufs=2)
                    for jt in range(NT):
                        nc.vector.tensor_scalar(
                            whp[:, jt * D:(jt + 1) * D],
                            wh[hh][:, jt * D:(jt + 1) * D],
                            s_rec[:, jt:jt + 1], None, AL.mult)

                    for ig in range(2):
                        pso = ps_O.tile([128, 8 * D], F32, tag="pso")
                        for k in range(8):
                            it2 = ig * 8 + k
                            for jt in range(NT):
                                nc.tensor.matmul(
                                    pso[:, k * D:(k + 1) * D],
                                    atT[hh][:, jt * N + it2 * 128: jt * N + (it2 + 1) * 128],
                                    whp[:, jt * D:(jt + 1) * D],
                                    start=(jt == 0), stop=(jt == NT - 1))
                        # ELU = relu(x) + (exp(min(x,0)) - 1)
                        hsb = po.tile([128, 8 * D], F32, tag="hsb", bufs=2)
                        nc.scalar.copy(hsb[:], pso[:])
                        tmin = po.tile([128, 8 * D], F32, tag="tmin", bufs=2)
                        nc.vector.tensor_scalar_min(tmin[:], hsb[:], 0.0)
                        texp = po.tile([128, 8 * D], F32, tag="texp", bufs=2)
                        nc.scalar.activation(texp[:], tmin[:], ACT.Exp, scale=1.0)
                        trelu = po.tile([128, 8 * D], F32, tag="trelu", bufs=2)
                        nc.vector.tensor_scalar_max(trelu[:], hsb[:], 0.0)
                        nc.vector.scalar_tensor_tensor(
                            out_sb[:].rearrange("p (t c) -> p t c", c=128)[
                                :, ig * 8:(ig + 1) * 8, hh * D:(hh + 1) * D],
                            texp[:], -1.0, trelu[:], AL.add, AL.add)

            # ---- final store ----
            nc.gpsimd.dma_start(
                out_d[:].rearrange("(t p) d -> p t d", p=128),
                out_sb[:].rearrange("p (t d) -> p t d", d=128))

    nc.compile()
    return nc


def _get_program():
    if "nc" not in _CACHE:
        _CACHE["nc"] = _build_program()
    return _CACHE["nc"]


def kernel(x, adj, type_onehot, Wq, Wk, W, a):
    from concourse.bass_utils import run_bass_kernel_spmd

    nc = _get_program()
    x = np.asarray(x, dtype=np.float32)
    adj = np.asarray(adj, dtype=np.int32)
    toh = np.asarray(type_onehot, dtype=np.float32)
    Wq = np.asarray(Wq, dtype=np.float32)
    Wk = np.asarray(Wk, dtype=np.float32)
    W = np.asarray(W, dtype=np.float32)
    a = np.asarray(a, dtype=np.float32)

    in_maps = []
    for c in range(8):
        b, hp = c // 2, c % 2
        hs = slice(2 * hp, 2 * hp + 2)
        in_maps.append({
            "adj": np.ascontiguousarray(adj[b]),
            "x": np.ascontiguousarray(x[b]),
            "toh": np.ascontiguousarray(toh[b]),
            "wq2": np.ascontiguousarray(Wq[hs]),
            "wk2": np.ascontiguousarray(Wk[hs]),
            "w2": np.ascontiguousarray(W[hs]),
            "a2": np.ascontiguousarray(a[hs]),
        })

    trace = bool(int(os.environ.get("GAT_TRACE", "0")))
    res = run_bass_kernel_spmd(nc, in_maps, core_ids=list(range(8)), trace=trace)
    _CACHE["last_result"] = res

    out = np.empty((B, N, H * D), dtype=np.float32)
    for c in range(8):
        b, hp = c // 2, c % 2
        out[b, :, 128 * hp:128 * (hp + 1)] = res.results[c]["out"]
    return out

